# revision 7
# baseline (speedup 1.0000x reference)
"""Trainium2 Bass kernel for nn_CDKANLayer (v3).

Computation (see problem reference):
  w_lag   = softmax(lag_logits, -1)                       [O,I,11]
  window  = x_history[:, T-11:T, :] reversed              [B,11,I]
  x_lagged[b,i,j] = sum_l window[b,l,j] * w_lag[i,j,l]
  xc      = clip(x_lagged, -1, 1)
  y_edge  = sum_c b_splines(xc) * coef                    (cubic B-spline)
  alpha   = sigmoid(mean_t(x_history)[b,j]*mod_w[i,j] + mod_b[i,j])
  out[b,i]= sum_j y_edge * alpha * sigmoid(adj_logits)[i,j]

v3 design (8 cores, shard in-features j; each core: 16 j x full B=256):
  - Spline as two-sided truncated-power cubic (as v2), but negative-side
    knots use s = min(x+t, 0), s^3 with the coefficient sign folded on
    host (kills the negx op).
  - Features live in ONE contiguous fp16 buffer per half (j 0-7 / 8-15):
    R = [r1|r2|s3|s4|xc] (5 blocks x [128,2048]); squares Q = R*R and
    cubes C = Q*R run as few big ops split across DVE/ACT/GpSimd.
  - x_lagged PSUM as 2x[128,2048] tiles -> clip is 2 big ops not 8.
  - combine: 8 accumulating diag matmuls per j on PE (incl. p0 via ones).
  - z = y*alpha as 8 pair-wide tensor_tensor ops; j-sum as a 4-op
    halving tree (replaces the 15-op serial GpSimd chain).
  - alpha in fp16; mean streams as fp8 matmuls (as v2).
  - DMA order: win/wlag first (lag MMs start ~1us), xh8 next, 4MB diag
    last in j-major order so combine chases the DMA.
"""

import os
import sys

import ml_dtypes
import numpy as np

for _p in ("/opt/trn_rl_repo", "/root/.axon_site/_ro/trn_rl_repo"):
    if os.path.isdir(_p) and _p not in sys.path:
        sys.path.insert(0, _p)

import concourse.bass as bass  # noqa: E402
import concourse.tile as tile  # noqa: E402
from concourse import bacc, mybir  # noqa: E402
from concourse import bass_utils  # noqa: E402

# ---------------------------------------------------------------- constants
B, T, I, O = 256, 512, 128, 128
L = 11                      # MAX_LAG + 1 lag taps
NCORES = 8
JC = I // NCORES            # j's per core = 16
JH = JC // 2                # j's per half = 8
HW = JH * B                 # half width in columns = 2048
GRID_SIZE, SPLINE_ORDER = 5, 3
GRID_LO, GRID_HI = -1.0, 1.0
H = (GRID_HI - GRID_LO) / GRID_SIZE
NP = 8                      # combine terms: 1, x, x2, x3, c1, c2, c3, c4

F32 = mybir.dt.float32
F16 = mybir.dt.float16
BF16 = mybir.dt.bfloat16
FP8 = mybir.dt.float8e4
ALU = mybir.AluOpType
ACTF = mybir.ActivationFunctionType

NP_F16 = np.float16
NP_BF16 = ml_dtypes.bfloat16
NP_FP8 = ml_dtypes.float8_e4m3

# feature-block order inside R / Q / C buffers
#   r1 = max(x-0.2, 0), r2 = max(x-0.6, 0)   (positive knots)
#   s3 = min(x+0.2, 0), s4 = min(x+0.6, 0)   (negative knots, sign folded)
#   xc = clip(x)
BLK = {"r1": 0, "r2": 1, "s3": 2, "s4": 3, "xc": 4}
NBLK = 5


# ------------------------------------------------------- host-side spline math
def _b_splines_np(x):
    """float64 copy of the reference b_splines (incl. its 1e-8 epsilons)."""
    g = (np.arange(-SPLINE_ORDER, GRID_SIZE + SPLINE_ORDER + 1, dtype=np.float64)
         * H + GRID_LO)
    x = np.asarray(x, dtype=np.float64)[..., None]
    bases = ((x >= g[:-1]) & (x < g[1:])).astype(np.float64)
    for i in range(1, SPLINE_ORDER + 1):
        t1 = (x - g[: -(i + 1)]) / (g[i:-1] - g[: -(i + 1)] + 1e-8) * bases[..., :-1]
        t2 = (g[i + 1:] - x) / (g[i + 1:] - g[1:-i] + 1e-8) * bases[..., 1:]
        bases = t1 + t2
    return bases


def _segment_poly_mats():
    """A[s] (4x8): on segment s, sum_c coef_c*B_c(x) = sum_d x^d*(A[s][d]@coef)."""
    mats = []
    for s in range(GRID_SIZE):
        lo = GRID_LO + s * H
        pts = lo + H * np.array([0.125, 0.375, 0.625, 0.875])
        Bm = _b_splines_np(pts)                       # [4, 8]
        V = np.vander(pts, 4, increasing=True)        # [4, 4]
        mats.append(np.linalg.solve(V, Bm))           # [4, 8]
    return np.stack(mats)                             # [5, 4, 8]


def _two_sided_params(coef64, mask):
    """[O, I, 8] float64: c0..c3 (center cubic), dR1,dR2,dL1',dL2'.

    Negative-knot coefficients carry the sign flip for the s=min form:
      dL*relu(-x-t)^3 == (-dL)*min(x+t,0)^3
    """
    Am = _segment_poly_mats()                          # [5,4,8]
    a = np.einsum("sdc,oic->sdoi", Am, coef64)         # [5,4,O,I]
    p = np.empty((O, I, NP), dtype=np.float64)
    p[..., 0:4] = np.moveaxis(a[2], 0, -1)             # center cubic c0..c3
    p[..., 4] = a[3, 3] - a[2, 3]                      # jump at +0.2
    p[..., 5] = a[4, 3] - a[3, 3]                      # jump at +0.6
    p[..., 6] = (a[1, 3] - a[2, 3])                    # knot -0.2 (s^3 form)
    p[..., 7] = (a[0, 3] - a[1, 3])                    # knot -0.6 (s^3 form)
    return p * mask[..., None]


def _host_precompute(x_history, coef, lag_logits, mod_w, mod_b, adj_logits):
    xh = np.asarray(x_history, dtype=np.float32)
    coef64 = np.asarray(coef, dtype=np.float64)
    ll = np.asarray(lag_logits, dtype=np.float64)

    m = ll.max(axis=-1, keepdims=True)
    e = np.exp(ll - m)
    w_lag = e / e.sum(axis=-1, keepdims=True)          # [O,I,L] f64

    mask = 1.0 / (1.0 + np.exp(-np.asarray(adj_logits, np.float64)[:O, :I]))
    params = _two_sided_params(coef64, mask)           # [O,I,8]

    window = xh[:, T - L:T, :][:, ::-1, :]             # [B,L,I]
    xh_tjb = np.ascontiguousarray(xh.transpose(1, 2, 0))  # [T, I, B]
    xh8_full = xh_tjb.astype(NP_FP8)

    rng = np.arange(128)
    in_maps = []
    for c in range(NCORES):
        sl = slice(c * JC, (c + 1) * JC)
        win = np.ascontiguousarray(
            window[:, :, sl].transpose(1, 2, 0)).astype(NP_BF16)   # [L,JC,B]
        wlg = np.ascontiguousarray(
            w_lag[:, sl, :].transpose(2, 1, 0)).astype(NP_BF16)    # [L,JC,O]
        xh8 = np.ascontiguousarray(xh8_full[:, sl, :])             # [T,JC,B]
        # diagonal combine tiles: [128 rows, j, p, 128 cols] j-major
        dg = np.zeros((128, JC, NP, 128), dtype=NP_F16)
        dg[rng, :, :, rng] = params[:, sl, :]                      # [O,JC,NP]
        sigsc = np.ascontiguousarray(
            np.asarray(mod_w, np.float64)[:, sl] / T).astype(np.float32)
        sigbi = np.ascontiguousarray(
            np.asarray(mod_b, np.float64)[:, sl]).astype(np.float32)
        in_maps.append({
            "win": win,
            "wlag": wlg,
            "xh8": xh8,
            "diag": np.ascontiguousarray(dg.reshape(128, NP * JC * 128)),
            "ones16": np.ones((128, B), dtype=NP_F16),
            "ones8": np.ones((128, 128), dtype=NP_FP8),
            "sigsc": sigsc,
            "sigbi": sigbi,
        })
    return in_maps


# ------------------------------------------------------------- device program
def _build_program():
    nc = bacc.Bacc("TRN2", target_bir_lowering=False, debug=False,
                   num_devices=NCORES)

    win_d = nc.dram_tensor("win", [L, JC, B], BF16, kind="ExternalInput")
    wlag_d = nc.dram_tensor("wlag", [L, JC, O], BF16, kind="ExternalInput")
    xh8_d = nc.dram_tensor("xh8", [T, JC, B], FP8, kind="ExternalInput")
    diag_d = nc.dram_tensor("diag", [128, NP * JC * 128], F16,
                            kind="ExternalInput")
    ones16_d = nc.dram_tensor("ones16", [128, B], F16, kind="ExternalInput")
    ones8_d = nc.dram_tensor("ones8", [128, 128], FP8, kind="ExternalInput")
    sigsc_d = nc.dram_tensor("sigsc", [O, JC], F32, kind="ExternalInput")
    sigbi_d = nc.dram_tensor("sigbi", [O, JC], F32, kind="ExternalInput")
    out_d = nc.dram_tensor("outp", [O, B], F32, kind="ExternalOutput")

    with tile.TileContext(nc) as tc:
        with (
            tc.tile_pool(name="pers", bufs=1) as pers,
            tc.tile_pool(name="xhp", bufs=4) as xhp,
            tc.tile_pool(name="psm", bufs=2, space=bass.MemorySpace.PSUM) as psm,
        ):
            # ---------------- persistent loads (order = DMA priority)
            win_sb = pers.tile([L, JC, B], BF16, tag="win")
            nc.sync.dma_start(win_sb[:], win_d[:])
            wlag_sb = pers.tile([L, JC, O], BF16, tag="wlag")
            nc.sync.dma_start(wlag_sb[:], wlag_d[:])
            ones16 = pers.tile([128, B], F16, tag="ones16")
            nc.sync.dma_start(ones16[:], ones16_d[:])
            ones8 = pers.tile([128, 128], FP8, tag="ones8")
            nc.sync.dma_start(ones8[:], ones8_d[:])
            sigsc = pers.tile([O, JC], F32, tag="sigsc")
            nc.sync.dma_start(sigsc[:], sigsc_d[:])
            sigbi = pers.tile([O, JC], F32, tag="sigbi")
            nc.sync.dma_start(sigbi[:], sigbi_d[:])

            # fp8 history: 8 tiles, each packing the 4 t-chunks of a j-pair
            xh_view = xh8_d.rearrange("(c p) j b -> p c (j b)", c=4)
            xts = []
            for r in range(8):
                xt = xhp.tile([128, 4, 512], FP8, tag="xh", name=f"xh{r}")
                nc.sync.dma_start(xt[:], xh_view[:, :, r * 512:(r + 1) * 512])
                xts.append(xt)

            # diagonal coef tiles, streamed in j order (combine chases this)
            diag = pers.tile([128, JC * NP * 128], F16, tag="diag")
            DGCH = NP * 128
            for j in range(JC):
                nc.sync.dma_start(diag[:, j * DGCH:(j + 1) * DGCH],
                                  diag_d[:, j * DGCH:(j + 1) * DGCH])

            def dg(p, j):
                off = (j * NP + p) * 128
                return diag[:, off:off + 128]

            # ---------------- PE: x_lagged (bf16, K=11) -> 2 big psum tiles
            ps_xl = []
            for h in range(2):                     # half h covers j = 8h..8h+7
                pt = psm.tile([128, JH * B], F32, tag="big", name=f"xl{h}")
                ps_xl.append(pt)
                for jl in range(JH):
                    ja = h * JH + jl
                    nc.tensor.matmul(pt[:, jl * B:(jl + 1) * B],
                                     wlag_sb[:, ja, :], win_sb[:, ja, :],
                                     start=True, stop=True)

            # ---------------- feature buffers: R/Q/C [128, 5 blocks x 2048]
            # per half; xc lives in block 4 of R (clip writes it directly).
            R = [pers.tile([128, NBLK * HW], F16, tag=f"R{h}", name=f"R{h}")
                 for h in range(2)]
            Q = [pers.tile([128, NBLK * HW], F16, tag=f"Q{h}", name=f"Q{h}")
                 for h in range(2)]
            C = [pers.tile([128, NBLK * HW], F16, tag=f"C{h}", name=f"C{h}")
                 for h in range(2)]

            def blk(buf, h, name, n=1):
                o = BLK[name] * HW
                return buf[h][:, o:o + n * HW]

            # ---------------- DVE: clip -> xc fp16 (one big op per half)
            for h in range(2):
                nc.vector.tensor_scalar(blk(R, h, "xc"), ps_xl[h][:],
                                        -1.0, 1.0, op0=ALU.max, op1=ALU.min)

            # ---------------- PE: mean stream (fp8, rows broadcast)
            # 2 big psum tiles; mean for j-pair r lands in 512-col slice.
            pmh = []
            for h in range(2):
                pm = psm.tile([128, JH * B], F32, tag="big", name=f"pm{h}")
                pmh.append(pm)
                for rr in range(4):
                    r = h * 4 + rr
                    for ch in range(4):
                        nc.tensor.matmul(pm[:, rr * 512:(rr + 1) * 512],
                                         ones8[:], xts[r][:, ch, :],
                                         start=(ch == 0), stop=(ch == 3))

            # ---------------- shifted clamps (r1,r2 on DVE; s3,s4 on GpSimd)
            for h in range(2):
                xc = blk(R, h, "xc")
                nc.vector.tensor_scalar(blk(R, h, "r1"), xc, -0.2, 0.0,
                                        op0=ALU.add, op1=ALU.max)
                nc.vector.tensor_scalar(blk(R, h, "r2"), xc, -0.6, 0.0,
                                        op0=ALU.add, op1=ALU.max)
                nc.gpsimd.tensor_scalar(blk(R, h, "s3"), xc, 0.2, 0.0,
                                        op0=ALU.add, op1=ALU.min)
                nc.gpsimd.tensor_scalar(blk(R, h, "s4"), xc, 0.6, 0.0,
                                        op0=ALU.add, op1=ALU.min)

            # ---------------- squares Q = R*R and cubes C = Q*R
            # split: ACT squares r1,r2 (2 blocks in one op per half);
            # DVE squares [s3|s4] + xc; cubes: DVE [r1|r2], GpSimd [s3|s4],
            # DVE xc (x3).
            for h in range(2):
                nc.scalar.activation(blk(Q, h, "r1", 2), blk(R, h, "r1", 2),
                                     ACTF.Square)
                nc.vector.tensor_tensor(blk(Q, h, "s3", 3), blk(R, h, "s3", 3),
                                        blk(R, h, "s3", 3), op=ALU.mult)
                nc.vector.tensor_tensor(blk(C, h, "r1", 2), blk(Q, h, "r1", 2),
                                        blk(R, h, "r1", 2), op=ALU.mult)
                nc.gpsimd.tensor_tensor(blk(C, h, "s3", 2), blk(Q, h, "s3", 2),
                                        blk(R, h, "s3", 2), op=ALU.mult)
                nc.vector.tensor_tensor(blk(C, h, "xc"), blk(Q, h, "xc"),
                                        blk(R, h, "xc"), op=ALU.mult)

            # ---------------- ACT: sigmoids from mean PSUM -> alpha fp16
            alpha = pers.tile([128, JC * B], F16, tag="alpha")
            for h in range(2):
                for jl in range(JH):
                    ja = h * JH + jl
                    nc.scalar.activation(
                        alpha[:, ja * B:(ja + 1) * B],
                        pmh[h][:, jl * B:(jl + 1) * B],
                        ACTF.Sigmoid, bias=sigbi[:, ja:ja + 1],
                        scale=sigsc[:, ja:ja + 1])

            # ---------------- PE: combine, 8 accumulating diag MMs per j
            # feature source per p-slot:
            #   0: ones, 1: xc(R), 2: x2(Q[xc]), 3: x3(C[xc]),
            #   4: C[r1], 5: C[r2], 6: C[s3], 7: C[s4]
            def feat(p, h, jl):
                sl = slice(jl * B, (jl + 1) * B)
                if p == 0:
                    return ones16[:]
                src = {1: (R, "xc"), 2: (Q, "xc"), 3: (C, "xc"),
                       4: (C, "r1"), 5: (C, "r2"), 6: (C, "s3"),
                       7: (C, "s4")}[p]
                return blk(src[0], h, src[1])[:, sl]

            zb = pers.tile([128, JC * B], F16, tag="zb")
            for h in range(2):                      # y for half h: j 8h..8h+7
                yt = psm.tile([128, JH * B], F32, tag="big", name=f"y{h}")
                for jl in range(JH):
                    ja = h * JH + jl
                    for pi, p in enumerate((0, 1, 2, 3, 4, 5, 6, 7)):
                        nc.tensor.matmul(yt[:, jl * B:(jl + 1) * B],
                                         dg(p, ja), feat(p, h, jl),
                                         start=(pi == 0), stop=(pi == 7))
                # z = y * alpha for the half (f32 PSUM x fp16 -> fp16)
                nc.vector.tensor_tensor(zb[:, h * HW:(h + 1) * HW],
                                        yt[:],
                                        alpha[:, h * HW:(h + 1) * HW],
                                        op=ALU.mult)

            # ---------------- j-sum: halving tree (contiguous adds)
            t1 = pers.tile([128, 2048], F16, tag="t1")
            t2 = pers.tile([128, 1024], F16, tag="t2")
            t3 = pers.tile([128, 512], F16, tag="t3")
            acc = pers.tile([128, B], F32, tag="acc")
            nc.gpsimd.tensor_tensor(t1[:], zb[:, 0:2048], zb[:, 2048:4096],
                                    op=ALU.add)
            nc.vector.tensor_tensor(t2[:], t1[:, 0:1024], t1[:, 1024:2048],
                                    op=ALU.add)
            nc.gpsimd.tensor_tensor(t3[:], t2[:, 0:512], t2[:, 512:1024],
                                    op=ALU.add)
            nc.vector.tensor_tensor(acc[:], t3[:, 0:B], t3[:, B:2 * B],
                                    op=ALU.add)

            nc.sync.dma_start(out_d[:], acc[:])

    nc.compile()
    return nc


_CACHED_NC = None


def _get_program():
    global _CACHED_NC
    if _CACHED_NC is None:
        _CACHED_NC = _build_program()
    return _CACHED_NC


# ------------------------------------------------------------------ entry
def kernel(x_history, coef, lag_logits, mod_w, mod_b, adj_logits):
    in_maps = _host_precompute(x_history, coef, lag_logits, mod_w, mod_b,
                               adj_logits)
    nc = _get_program()
    res = bass_utils.run_bass_kernel_spmd(nc, in_maps,
                                          core_ids=list(range(NCORES)))
    total = np.zeros((O, B), dtype=np.float64)
    for c in range(NCORES):
        total += np.asarray(res.results[c]["outp"], dtype=np.float64)
    return np.ascontiguousarray(total.T.astype(np.float32))


# -------------------------------------------- pure-numpy emulation (testing)
def emulate(x_history, coef, lag_logits, mod_w, mod_b, adj_logits):
    """Numpy mirror of the v3 device algorithm (f32-ish, no dtype sim)."""
    in_maps = _host_precompute(x_history, coef, lag_logits, mod_w, mod_b,
                               adj_logits)
    total = np.zeros((O, B), dtype=np.float64)
    for c in range(NCORES):
        total += emulate_core(in_maps[c])
    return total.T.astype(np.float32)


def emulate_core(im):
    win = im["win"].astype(np.float64)            # [L,JC,B]
    wlg = im["wlag"].astype(np.float64)           # [L,JC,O]
    dgf = im["diag"].astype(np.float64).reshape(128, JC, NP, 128)
    params = dgf[np.arange(128), :, :, np.arange(128)]   # [128,JC,NP] (o,j,p)
    params = params.transpose(0, 2, 1)                   # [128,NP,JC]
    xm = im["xh8"].astype(np.float64).mean(axis=0)       # [JC,B]
    sigsc = im["sigsc"].astype(np.float64)        # [O,JC]
    sigbi = im["sigbi"].astype(np.float64)

    part = np.zeros((O, B), dtype=np.float64)
    for jl in range(JC):
        xl = wlg[:, jl, :].T @ win[:, jl, :]      # [O,B]
        x = np.clip(xl, -1.0, 1.0)
        f = [np.ones_like(x), x, x * x, x ** 3,
             np.maximum(x - 0.2, 0) ** 3, np.maximum(x - 0.6, 0) ** 3,
             np.minimum(x + 0.2, 0) ** 3, np.minimum(x + 0.6, 0) ** 3]
        y = np.zeros_like(x)
        for p in range(NP):
            y += params[:, p, jl][:, None] * f[p]
        lin = sigsc[:, jl][:, None] * (xm[jl] * T)[None, :] + sigbi[:, jl][:, None]
        part += y / (1.0 + np.exp(-lin))
    return part


# revision 14
# speedup vs baseline: 2.9738x; 2.9738x over previous
"""Trainium2 Bass kernel for nn_CDKANLayer (v3.1).

Computation (see problem reference):
  w_lag   = softmax(lag_logits, -1)                       [O,I,11]
  window  = x_history[:, T-11:T, :] reversed              [B,11,I]
  x_lagged[b,i,j] = sum_l window[b,l,j] * w_lag[i,j,l]
  xc      = clip(x_lagged, -1, 1)
  y_edge  = sum_c b_splines(xc) * coef                    (cubic B-spline)
  alpha   = sigmoid(mean_t(x_history)[b,j]*mod_w[i,j] + mod_b[i,j])
  out[b,i]= sum_j y_edge * alpha * sigmoid(adj_logits)[i,j]

v3.1 design (8 cores, shard in-features j; each core: 16 j x full B=256):
  - Two-sided truncated-power cubic (v2 param): features 1, x, x2, x3,
    r1^3, r2^3, r3^3, r4^3 with r = relu(+-x - t), negative-side signs
    folded into the host coefficients.
  - Combine on PE as accumulating diag matmuls, split into phase A
    (ones/x/x2/x3 — available early) and phase B (the four cubes), with
    interleaved PSUM groups across the 8 banks so PE never waits on the
    cube chain.
  - Feature ops at half-j granularity ([128,2048]) to shorten the
    clip->shift->square->cube critical path; split across DVE and ACT.
    GpSimd only does the small late j-sum tree (it cannot touch PSUM and
    big GpSimd ops destroy DVE throughput via shared SBUF ports).
  - z = y*alpha per j-pair on DVE; j-sum as halving tree; alpha fp16.
  - PSUM: quarter tiles [128,1024] ring (lag then mean) + 4 y banks.
"""

import os
import sys

import ml_dtypes
import numpy as np

for _p in ("/opt/trn_rl_repo", "/root/.axon_site/_ro/trn_rl_repo"):
    if os.path.isdir(_p) and _p not in sys.path:
        sys.path.insert(0, _p)

import concourse.bass as bass  # noqa: E402
import concourse.tile as tile  # noqa: E402
from concourse import bacc, mybir  # noqa: E402
from concourse import bass_utils  # noqa: E402

# ---------------------------------------------------------------- constants
B, T, I, O = 256, 512, 128, 128
L = 11                      # MAX_LAG + 1 lag taps
NCORES = 8
JC = I // NCORES            # j's per core = 16
JH = JC // 2                # j's per half = 8
JQ = JC // 4                # j's per quarter = 4
HW = JH * B                 # half width in columns = 2048
QW = JQ * B                 # quarter width = 1024
GRID_SIZE, SPLINE_ORDER = 5, 3
GRID_LO, GRID_HI = -1.0, 1.0
H = (GRID_HI - GRID_LO) / GRID_SIZE
NP = 8                      # combine terms: 1, x, x2, x3, c1, c2, c3, c4

F32 = mybir.dt.float32
F16 = mybir.dt.float16
BF16 = mybir.dt.bfloat16
FP8 = mybir.dt.float8e4
ALU = mybir.AluOpType
ACTF = mybir.ActivationFunctionType

NP_F16 = np.float16
NP_BF16 = ml_dtypes.bfloat16
NP_FP8 = ml_dtypes.float8_e4m3

# feature-block order inside R / Q / C buffers (per half):
#   r1 = relu(x-0.2), r2 = relu(x-0.6), r3 = relu(-x-0.2), r4 = relu(-x-0.6)
#   xc = clip(x)  (basis for x2/x3 in Q/C)
BLK = {"r1": 0, "r2": 1, "r3": 2, "r4": 3, "xc": 4}
NBLK = 5


# ------------------------------------------------------- host-side spline math
def _b_splines_np(x):
    """float64 copy of the reference b_splines (incl. its 1e-8 epsilons)."""
    g = (np.arange(-SPLINE_ORDER, GRID_SIZE + SPLINE_ORDER + 1, dtype=np.float64)
         * H + GRID_LO)
    x = np.asarray(x, dtype=np.float64)[..., None]
    bases = ((x >= g[:-1]) & (x < g[1:])).astype(np.float64)
    for i in range(1, SPLINE_ORDER + 1):
        t1 = (x - g[: -(i + 1)]) / (g[i:-1] - g[: -(i + 1)] + 1e-8) * bases[..., :-1]
        t2 = (g[i + 1:] - x) / (g[i + 1:] - g[1:-i] + 1e-8) * bases[..., 1:]
        bases = t1 + t2
    return bases


def _segment_poly_mats():
    """A[s] (4x8): on segment s, sum_c coef_c*B_c(x) = sum_d x^d*(A[s][d]@coef)."""
    mats = []
    for s in range(GRID_SIZE):
        lo = GRID_LO + s * H
        pts = lo + H * np.array([0.125, 0.375, 0.625, 0.875])
        Bm = _b_splines_np(pts)                       # [4, 8]
        V = np.vander(pts, 4, increasing=True)        # [4, 4]
        mats.append(np.linalg.solve(V, Bm))           # [4, 8]
    return np.stack(mats)                             # [5, 4, 8]


def _two_sided_params(coef64, mask):
    """[O, I, 8] float64: c0..c3 (center cubic), dR1,dR2,dL1,dL2 (r-form)."""
    Am = _segment_poly_mats()                          # [5,4,8]
    a = np.einsum("sdc,oic->sdoi", Am, coef64)         # [5,4,O,I]
    p = np.empty((O, I, NP), dtype=np.float64)
    p[..., 0:4] = np.moveaxis(a[2], 0, -1)             # center cubic c0..c3
    p[..., 4] = a[3, 3] - a[2, 3]                      # jump at +0.2
    p[..., 5] = a[4, 3] - a[3, 3]                      # jump at +0.6
    p[..., 6] = -(a[1, 3] - a[2, 3])                   # knot -0.2, relu(-x-.2)^3
    p[..., 7] = -(a[0, 3] - a[1, 3])                   # knot -0.6, relu(-x-.6)^3
    return p * mask[..., None]


def _host_precompute(x_history, coef, lag_logits, mod_w, mod_b, adj_logits):
    xh = np.asarray(x_history, dtype=np.float32)
    coef64 = np.asarray(coef, dtype=np.float64)
    ll = np.asarray(lag_logits, dtype=np.float64)

    m = ll.max(axis=-1, keepdims=True)
    e = np.exp(ll - m)
    w_lag = e / e.sum(axis=-1, keepdims=True)          # [O,I,L] f64

    mask = 1.0 / (1.0 + np.exp(-np.asarray(adj_logits, np.float64)[:O, :I]))
    params = _two_sided_params(coef64, mask)           # [O,I,8]

    window = xh[:, T - L:T, :][:, ::-1, :]             # [B,L,I]
    xh_tjb = np.ascontiguousarray(xh.transpose(1, 2, 0))  # [T, I, B]
    xh8_full = xh_tjb.astype(NP_FP8)

    rng = np.arange(128)
    in_maps = []
    for c in range(NCORES):
        sl = slice(c * JC, (c + 1) * JC)
        win = np.ascontiguousarray(
            window[:, :, sl].transpose(1, 2, 0)).astype(NP_BF16)   # [L,JC,B]
        wlg = np.ascontiguousarray(
            w_lag[:, sl, :].transpose(2, 1, 0)).astype(NP_BF16)    # [L,JC,O]
        xh8 = np.ascontiguousarray(xh8_full[:, sl, :])             # [T,JC,B]
        # diagonal combine tiles: [128 rows, j, p, 128 cols] j-major
        dg = np.zeros((128, JC, NP, 128), dtype=NP_F16)
        dg[rng, :, :, rng] = params[:, sl, :]                      # [O,JC,NP]
        sigsc = np.ascontiguousarray(
            np.asarray(mod_w, np.float64)[:, sl] / T).astype(np.float32)
        sigbi = np.ascontiguousarray(
            np.asarray(mod_b, np.float64)[:, sl]).astype(np.float32)
        in_maps.append({
            "win": win,
            "wlag": wlg,
            "xh8": xh8,
            "diag": np.ascontiguousarray(dg.reshape(128, NP * JC * 128)),
            "ones16": np.ones((128, B), dtype=NP_F16),
            "ones8": np.ones((128, 128), dtype=NP_FP8),
            "sigsc": sigsc,
            "sigbi": sigbi,
        })
    return in_maps


# ------------------------------------------------------------- device program
def _build_program():
    nc = bacc.Bacc("TRN2", target_bir_lowering=False, debug=False,
                   num_devices=NCORES)

    win_d = nc.dram_tensor("win", [L, JC, B], BF16, kind="ExternalInput")
    wlag_d = nc.dram_tensor("wlag", [L, JC, O], BF16, kind="ExternalInput")
    xh8_d = nc.dram_tensor("xh8", [T, JC, B], FP8, kind="ExternalInput")
    diag_d = nc.dram_tensor("diag", [128, NP * JC * 128], F16,
                            kind="ExternalInput")
    ones16_d = nc.dram_tensor("ones16", [128, B], F16, kind="ExternalInput")
    ones8_d = nc.dram_tensor("ones8", [128, 128], FP8, kind="ExternalInput")
    sigsc_d = nc.dram_tensor("sigsc", [O, JC], F32, kind="ExternalInput")
    sigbi_d = nc.dram_tensor("sigbi", [O, JC], F32, kind="ExternalInput")
    out_d = nc.dram_tensor("outp", [O, B], F32, kind="ExternalOutput")

    with tile.TileContext(nc) as tc:
        with (
            tc.tile_pool(name="pers", bufs=1) as pers,
            tc.tile_pool(name="xhp", bufs=4) as xhp,
            tc.tile_pool(name="psq", bufs=2, space=bass.MemorySpace.PSUM) as psq,
            tc.tile_pool(name="psy", bufs=4, space=bass.MemorySpace.PSUM) as psy,
        ):
            # ---------------- persistent loads (order = DMA priority)
            win_sb = pers.tile([L, JC, B], BF16, tag="win")
            nc.sync.dma_start(win_sb[:], win_d[:])
            wlag_sb = pers.tile([L, JC, O], BF16, tag="wlag")
            nc.sync.dma_start(wlag_sb[:], wlag_d[:])
            ones16 = pers.tile([128, B], F16, tag="ones16")
            nc.sync.dma_start(ones16[:], ones16_d[:])
            ones8 = pers.tile([128, 128], FP8, tag="ones8")
            nc.sync.dma_start(ones8[:], ones8_d[:])
            sigsc = pers.tile([O, JC], F32, tag="sigsc")
            nc.sync.dma_start(sigsc[:], sigsc_d[:])
            sigbi = pers.tile([O, JC], F32, tag="sigbi")
            nc.sync.dma_start(sigbi[:], sigbi_d[:])

            # per-partition bias constants for the ACT Relu shifts
            bneg2 = pers.tile([128, 1], F32, tag="bneg2")
            nc.gpsimd.memset(bneg2[:], -0.2)
            bneg6 = pers.tile([128, 1], F32, tag="bneg6")
            nc.gpsimd.memset(bneg6[:], -0.6)

            # fp8 history: 8 tiles, each packing the 4 t-chunks of a j-pair
            xh_view = xh8_d.rearrange("(c p) j b -> p c (j b)", c=4)
            xts = []
            for r in range(8):
                xt = xhp.tile([128, 4, 512], FP8, tag="xh", name=f"xh{r}")
                nc.sync.dma_start(xt[:], xh_view[:, :, r * 512:(r + 1) * 512])
                xts.append(xt)

            # diagonal coef tiles, streamed in j order (combine chases this)
            diag = pers.tile([128, JC * NP * 128], F16, tag="diag")
            DGCH = NP * 128
            for j in range(JC):
                nc.sync.dma_start(diag[:, j * DGCH:(j + 1) * DGCH],
                                  diag_d[:, j * DGCH:(j + 1) * DGCH])

            def dg(p, j):
                off = (j * NP + p) * 128
                return diag[:, off:off + 128]

            # ---------------- feature buffers per half: [128, 5 x 2048] fp16
            R = [pers.tile([128, NBLK * HW], F16, tag=f"R{h}", name=f"R{h}")
                 for h in range(2)]
            Q = [pers.tile([128, NBLK * HW], F16, tag=f"Q{h}", name=f"Q{h}")
                 for h in range(2)]
            C = [pers.tile([128, NBLK * HW], F16, tag=f"C{h}", name=f"C{h}")
                 for h in range(2)]

            def blk(buf, h, name, n=1, q=None):
                o = BLK[name] * HW
                if q is not None:            # quarter slice within the block
                    o += (q % 2) * QW
                    return buf[h][:, o:o + QW]
                return buf[h][:, o:o + n * HW]

            # ---------------- PE: x_lagged (bf16, K=11) -> quarter psum ring
            # quarter qq covers j = 4qq..4qq+3; clip chases each quarter.
            for qq in range(4):
                pt = psq.tile([128, QW], F32, tag="q", name=f"xl{qq}")
                for jl in range(JQ):
                    ja = qq * JQ + jl
                    nc.tensor.matmul(pt[:, jl * B:(jl + 1) * B],
                                     wlag_sb[:, ja, :], win_sb[:, ja, :],
                                     start=True, stop=True)
                h = qq // 2
                nc.vector.tensor_scalar(blk(R, h, "xc", q=qq), pt[:],
                                        -1.0, 1.0, op0=ALU.max, op1=ALU.min)

            # ---------------- PE: mean stream (fp8, rows broadcast), quarters
            pms = []
            for qq in range(4):
                pm = psq.tile([128, QW], F32, tag="q", name=f"pm{qq}")
                pms.append(pm)
                for rr in range(2):
                    r = qq * 2 + rr
                    for ch in range(4):
                        nc.tensor.matmul(pm[:, rr * 512:(rr + 1) * 512],
                                         ones8[:], xts[r][:, ch, :],
                                         start=(ch == 0), stop=(ch == 3))

            # ---------------- features per half h (after clip quarters)
            for h in range(2):
                xc = blk(R, h, "xc")
                # shifts: r1,r2 on DVE; r3,r4 on ACT (Relu with scale=-1)
                nc.vector.tensor_scalar(blk(R, h, "r1"), xc, -0.2, 0.0,
                                        op0=ALU.add, op1=ALU.max)
                nc.vector.tensor_scalar(blk(R, h, "r2"), xc, -0.6, 0.0,
                                        op0=ALU.add, op1=ALU.max)
                negx = blk(Q, h, "r3")      # scratch (overwritten below)
                nc.vector.tensor_scalar(negx, xc, -1.0, None, op0=ALU.mult)
                nc.vector.tensor_scalar(blk(R, h, "r3"), negx, -0.2, 0.0,
                                        op0=ALU.add, op1=ALU.max)
                nc.vector.tensor_scalar(blk(R, h, "r4"), negx, -0.6, 0.0,
                                        op0=ALU.add, op1=ALU.max)
                # squares: x2 on ACT (early, feeds combine phase A);
                # q[r1|r2] on ACT; q[r3|r4] on DVE
                nc.scalar.activation(blk(Q, h, "xc"), xc, ACTF.Square)
                nc.vector.tensor_tensor(blk(C, h, "xc"), blk(Q, h, "xc"), xc,
                                        op=ALU.mult)          # x3 (early)
                nc.scalar.activation(blk(Q, h, "r1", 2), blk(R, h, "r1", 2),
                                     ACTF.Square)
                nc.vector.tensor_tensor(blk(Q, h, "r3", 2), blk(R, h, "r3", 2),
                                        blk(R, h, "r3", 2), op=ALU.mult)
                # cubes
                nc.vector.tensor_tensor(blk(C, h, "r1", 2), blk(Q, h, "r1", 2),
                                        blk(R, h, "r1", 2), op=ALU.mult)
                nc.vector.tensor_tensor(blk(C, h, "r3", 2), blk(Q, h, "r3", 2),
                                        blk(R, h, "r3", 2), op=ALU.mult)

            # ---------------- ACT: sigmoids from mean PSUM -> alpha fp16
            alpha = pers.tile([128, JC * B], F16, tag="alpha")
            for qq in range(4):
                for jl in range(JQ):
                    ja = qq * JQ + jl
                    nc.scalar.activation(
                        alpha[:, ja * B:(ja + 1) * B],
                        pms[qq][:, jl * B:(jl + 1) * B],
                        ACTF.Sigmoid, bias=sigbi[:, ja:ja + 1],
                        scale=sigsc[:, ja:ja + 1])

            # ---------------- PE: combine (phase A then phase B) + z
            # p -> feature: 0 ones, 1 xc, 2 x2 (Q[xc]), 3 x3 (C[xc]),
            #               4 C[r1], 5 C[r2], 6 C[r3], 7 C[r4]
            def feat(p, ja):
                h, jl = ja // JH, ja % JH
                sl = slice(jl * B, (jl + 1) * B)
                if p == 0:
                    return ones16[:]
                src = {1: (R, "xc"), 2: (Q, "xc"), 3: (C, "xc"),
                       4: (C, "r1"), 5: (C, "r2"), 6: (C, "r3"),
                       7: (C, "r4")}[p]
                return blk(src[0], h, src[1])[:, sl]

            zb = pers.tile([128, JC * B], F16, tag="zb")
            for t in range(8):                      # j-pair per bank
                yt = psy.tile([128, 512], F32, tag="y", name=f"y{t}")
                for hh in range(2):                 # contiguous 8-MM group
                    ja = 2 * t + hh
                    for p in range(8):
                        nc.tensor.matmul(yt[:, hh * B:(hh + 1) * B],
                                         dg(p, ja), feat(p, ja),
                                         start=(p == 0), stop=(p == 7))
                nc.vector.tensor_tensor(
                    zb[:, t * 512:(t + 1) * 512], yt[:],
                    alpha[:, t * 512:(t + 1) * 512], op=ALU.mult)

            # ---------------- j-sum: halving tree (small, on GpSimd + DVE)
            t1 = pers.tile([128, 2048], F16, tag="t1")
            t2 = pers.tile([128, 1024], F16, tag="t2")
            t3 = pers.tile([128, 512], F16, tag="t3")
            acc = pers.tile([128, B], F32, tag="acc")
            nc.gpsimd.tensor_tensor(t1[:], zb[:, 0:2048], zb[:, 2048:4096],
                                    op=ALU.add)
            nc.vector.tensor_tensor(t2[:], t1[:, 0:1024], t1[:, 1024:2048],
                                    op=ALU.add)
            nc.gpsimd.tensor_tensor(t3[:], t2[:, 0:512], t2[:, 512:1024],
                                    op=ALU.add)
            nc.vector.tensor_tensor(acc[:], t3[:, 0:B], t3[:, B:2 * B],
                                    op=ALU.add)

            nc.sync.dma_start(out_d[:], acc[:])

    nc.compile()
    return nc


_CACHED_NC = None


def _get_program():
    global _CACHED_NC
    if _CACHED_NC is None:
        _CACHED_NC = _build_program()
    return _CACHED_NC


# ------------------------------------------------------------------ entry
def kernel(x_history, coef, lag_logits, mod_w, mod_b, adj_logits):
    in_maps = _host_precompute(x_history, coef, lag_logits, mod_w, mod_b,
                               adj_logits)
    nc = _get_program()
    res = bass_utils.run_bass_kernel_spmd(nc, in_maps,
                                          core_ids=list(range(NCORES)))
    total = np.zeros((O, B), dtype=np.float64)
    for c in range(NCORES):
        total += np.asarray(res.results[c]["outp"], dtype=np.float64)
    return np.ascontiguousarray(total.T.astype(np.float32))


# -------------------------------------------- pure-numpy emulation (testing)
def emulate(x_history, coef, lag_logits, mod_w, mod_b, adj_logits):
    """Numpy mirror of the v3.1 device algorithm (f32-ish, no dtype sim)."""
    in_maps = _host_precompute(x_history, coef, lag_logits, mod_w, mod_b,
                               adj_logits)
    total = np.zeros((O, B), dtype=np.float64)
    for c in range(NCORES):
        total += emulate_core(in_maps[c])
    return total.T.astype(np.float32)


def emulate_core(im):
    win = im["win"].astype(np.float64)            # [L,JC,B]
    wlg = im["wlag"].astype(np.float64)           # [L,JC,O]
    dgf = im["diag"].astype(np.float64).reshape(128, JC, NP, 128)
    params = dgf[np.arange(128), :, :, np.arange(128)]   # [128,JC,NP] (o,j,p)
    params = params.transpose(0, 2, 1)                   # [128,NP,JC]
    xm = im["xh8"].astype(np.float64).mean(axis=0)       # [JC,B]
    sigsc = im["sigsc"].astype(np.float64)        # [O,JC]
    sigbi = im["sigbi"].astype(np.float64)

    part = np.zeros((O, B), dtype=np.float64)
    for jl in range(JC):
        xl = wlg[:, jl, :].T @ win[:, jl, :]      # [O,B]
        x = np.clip(xl, -1.0, 1.0)
        f = [np.ones_like(x), x, x * x, x ** 3,
             np.maximum(x - 0.2, 0) ** 3, np.maximum(x - 0.6, 0) ** 3,
             np.maximum(-x - 0.2, 0) ** 3, np.maximum(-x - 0.6, 0) ** 3]
        y = np.zeros_like(x)
        for p in range(NP):
            y += params[:, p, jl][:, None] * f[p]
        lin = sigsc[:, jl][:, None] * (xm[jl] * T)[None, :] + sigbi[:, jl][:, None]
        part += y / (1.0 + np.exp(-lin))
    return part


# revision 17
# speedup vs baseline: 3.2243x; 1.0842x over previous
"""Trainium2 Bass kernel for nn_CDKANLayer (v3.1).

Computation (see problem reference):
  w_lag   = softmax(lag_logits, -1)                       [O,I,11]
  window  = x_history[:, T-11:T, :] reversed              [B,11,I]
  x_lagged[b,i,j] = sum_l window[b,l,j] * w_lag[i,j,l]
  xc      = clip(x_lagged, -1, 1)
  y_edge  = sum_c b_splines(xc) * coef                    (cubic B-spline)
  alpha   = sigmoid(mean_t(x_history)[b,j]*mod_w[i,j] + mod_b[i,j])
  out[b,i]= sum_j y_edge * alpha * sigmoid(adj_logits)[i,j]

v3.1 design (8 cores, shard in-features j; each core: 16 j x full B=256):
  - Two-sided truncated-power cubic (v2 param): features 1, x, x2, x3,
    r1^3, r2^3, r3^3, r4^3 with r = relu(+-x - t), negative-side signs
    folded into the host coefficients.
  - Combine on PE as accumulating diag matmuls, split into phase A
    (ones/x/x2/x3 — available early) and phase B (the four cubes), with
    interleaved PSUM groups across the 8 banks so PE never waits on the
    cube chain.
  - Feature ops at half-j granularity ([128,2048]) to shorten the
    clip->shift->square->cube critical path; split across DVE and ACT.
    GpSimd only does the small late j-sum tree (it cannot touch PSUM and
    big GpSimd ops destroy DVE throughput via shared SBUF ports).
  - z = y*alpha per j-pair on DVE; j-sum as halving tree; alpha fp16.
  - PSUM: quarter tiles [128,1024] ring (lag then mean) + 4 y banks.
"""

import os
import sys

import ml_dtypes
import numpy as np

for _p in ("/opt/trn_rl_repo", "/root/.axon_site/_ro/trn_rl_repo"):
    if os.path.isdir(_p) and _p not in sys.path:
        sys.path.insert(0, _p)

import concourse.bass as bass  # noqa: E402
import concourse.tile as tile  # noqa: E402
from concourse import bacc, mybir  # noqa: E402
from concourse import bass_utils  # noqa: E402

# ---------------------------------------------------------------- constants
B, T, I, O = 256, 512, 128, 128
L = 11                      # MAX_LAG + 1 lag taps
NCORES = 8
JC = I // NCORES            # j's per core = 16
JH = JC // 2                # j's per half = 8
JQ = JC // 4                # j's per quarter = 4
HW = JH * B                 # half width in columns = 2048
QW = JQ * B                 # quarter width = 1024
GRID_SIZE, SPLINE_ORDER = 5, 3
GRID_LO, GRID_HI = -1.0, 1.0
H = (GRID_HI - GRID_LO) / GRID_SIZE
NP = 8                      # combine terms: 1, x, x2, x3, c1, c2, c3, c4

F32 = mybir.dt.float32
F16 = mybir.dt.float16
BF16 = mybir.dt.bfloat16
FP8 = mybir.dt.float8e4
ALU = mybir.AluOpType
ACTF = mybir.ActivationFunctionType

NP_F16 = np.float16
NP_BF16 = ml_dtypes.bfloat16
NP_FP8 = ml_dtypes.float8_e4m3

# feature-block order inside R / Q / C buffers (per half):
#   r1 = relu(x-0.2), r2 = relu(x-0.6), r3 = relu(-x-0.2), r4 = relu(-x-0.6)
#   xc = clip(x)  (basis for x2/x3 in Q/C)
BLK = {"r1": 0, "r2": 1, "r3": 2, "r4": 3, "xc": 4}
NBLK = 5


# ------------------------------------------------------- host-side spline math
def _b_splines_np(x):
    """float64 copy of the reference b_splines (incl. its 1e-8 epsilons)."""
    g = (np.arange(-SPLINE_ORDER, GRID_SIZE + SPLINE_ORDER + 1, dtype=np.float64)
         * H + GRID_LO)
    x = np.asarray(x, dtype=np.float64)[..., None]
    bases = ((x >= g[:-1]) & (x < g[1:])).astype(np.float64)
    for i in range(1, SPLINE_ORDER + 1):
        t1 = (x - g[: -(i + 1)]) / (g[i:-1] - g[: -(i + 1)] + 1e-8) * bases[..., :-1]
        t2 = (g[i + 1:] - x) / (g[i + 1:] - g[1:-i] + 1e-8) * bases[..., 1:]
        bases = t1 + t2
    return bases


def _segment_poly_mats():
    """A[s] (4x8): on segment s, sum_c coef_c*B_c(x) = sum_d x^d*(A[s][d]@coef)."""
    mats = []
    for s in range(GRID_SIZE):
        lo = GRID_LO + s * H
        pts = lo + H * np.array([0.125, 0.375, 0.625, 0.875])
        Bm = _b_splines_np(pts)                       # [4, 8]
        V = np.vander(pts, 4, increasing=True)        # [4, 4]
        mats.append(np.linalg.solve(V, Bm))           # [4, 8]
    return np.stack(mats)                             # [5, 4, 8]


def _two_sided_params(coef64, mask):
    """[O, I, 8] float64: c0..c3 (center cubic), dR1,dR2,dL1,dL2 (r-form)."""
    Am = _segment_poly_mats()                          # [5,4,8]
    a = np.einsum("sdc,oic->sdoi", Am, coef64)         # [5,4,O,I]
    p = np.empty((O, I, NP), dtype=np.float64)
    p[..., 0:4] = np.moveaxis(a[2], 0, -1)             # center cubic c0..c3
    p[..., 4] = a[3, 3] - a[2, 3]                      # jump at +0.2
    p[..., 5] = a[4, 3] - a[3, 3]                      # jump at +0.6
    p[..., 6] = -(a[1, 3] - a[2, 3])                   # knot -0.2, relu(-x-.2)^3
    p[..., 7] = -(a[0, 3] - a[1, 3])                   # knot -0.6, relu(-x-.6)^3
    return p * mask[..., None]


def _host_precompute(x_history, coef, lag_logits, mod_w, mod_b, adj_logits):
    xh = np.asarray(x_history, dtype=np.float32)
    coef64 = np.asarray(coef, dtype=np.float64)
    ll = np.asarray(lag_logits, dtype=np.float64)

    m = ll.max(axis=-1, keepdims=True)
    e = np.exp(ll - m)
    w_lag = e / e.sum(axis=-1, keepdims=True)          # [O,I,L] f64

    mask = 1.0 / (1.0 + np.exp(-np.asarray(adj_logits, np.float64)[:O, :I]))
    params = _two_sided_params(coef64, mask)           # [O,I,8]

    window = xh[:, T - L:T, :][:, ::-1, :]             # [B,L,I]
    xh_tjb = np.ascontiguousarray(xh.transpose(1, 2, 0))  # [T, I, B]
    xh8_full = xh_tjb.astype(NP_FP8)

    rng = np.arange(128)
    in_maps = []
    for c in range(NCORES):
        sl = slice(c * JC, (c + 1) * JC)
        win = np.ascontiguousarray(
            window[:, :, sl].transpose(1, 2, 0)).astype(NP_BF16)   # [L,JC,B]
        wlg = np.ascontiguousarray(
            w_lag[:, sl, :].transpose(2, 1, 0)).astype(NP_BF16)    # [L,JC,O]
        xh8 = np.ascontiguousarray(xh8_full[:, sl, :])             # [T,JC,B]
        # diagonal combine tiles: [128 rows, j, p, 128 cols] j-major
        dg = np.zeros((128, JC, NP, 128), dtype=NP_F16)
        dg[rng, :, :, rng] = params[:, sl, :]                      # [O,JC,NP]
        sigsc = np.ascontiguousarray(
            np.asarray(mod_w, np.float64)[:, sl] / T).astype(np.float32)
        sigbi = np.ascontiguousarray(
            np.asarray(mod_b, np.float64)[:, sl]).astype(np.float32)
        in_maps.append({
            "win": win,
            "wlag": wlg,
            "xh8": xh8,
            "diag": np.ascontiguousarray(dg.reshape(128, NP * JC * 128)),
            "ones16": np.ones((128, B), dtype=NP_F16),
            "ones8": np.ones((128, 128), dtype=NP_FP8),
            "sigsc": sigsc,
            "sigbi": sigbi,
        })
    return in_maps


# ------------------------------------------------------------- device program
def _build_program():
    nc = bacc.Bacc("TRN2", target_bir_lowering=False, debug=False,
                   num_devices=NCORES)

    win_d = nc.dram_tensor("win", [L, JC, B], BF16, kind="ExternalInput")
    wlag_d = nc.dram_tensor("wlag", [L, JC, O], BF16, kind="ExternalInput")
    xh8_d = nc.dram_tensor("xh8", [T, JC, B], FP8, kind="ExternalInput")
    diag_d = nc.dram_tensor("diag", [128, NP * JC * 128], F16,
                            kind="ExternalInput")
    ones16_d = nc.dram_tensor("ones16", [128, B], F16, kind="ExternalInput")
    ones8_d = nc.dram_tensor("ones8", [128, 128], FP8, kind="ExternalInput")
    sigsc_d = nc.dram_tensor("sigsc", [O, JC], F32, kind="ExternalInput")
    sigbi_d = nc.dram_tensor("sigbi", [O, JC], F32, kind="ExternalInput")
    out_d = nc.dram_tensor("outp", [O, B], F32, kind="ExternalOutput")

    with tile.TileContext(nc) as tc:
        with (
            tc.tile_pool(name="pers", bufs=1) as pers,
            tc.tile_pool(name="xhp", bufs=4) as xhp,
            tc.tile_pool(name="psq", bufs=2, space=bass.MemorySpace.PSUM) as psq,
            tc.tile_pool(name="psy", bufs=2, space=bass.MemorySpace.PSUM) as psy,
        ):
            # ---------------- persistent loads (order = DMA priority)
            win_sb = pers.tile([L, JC, B], BF16, tag="win")
            nc.sync.dma_start(win_sb[:], win_d[:])
            wlag_sb = pers.tile([L, JC, O], BF16, tag="wlag")
            nc.sync.dma_start(wlag_sb[:], wlag_d[:])
            ones16 = pers.tile([128, B], F16, tag="ones16")
            nc.sync.dma_start(ones16[:], ones16_d[:])
            ones8 = pers.tile([128, 128], FP8, tag="ones8")
            nc.sync.dma_start(ones8[:], ones8_d[:])
            sigsc = pers.tile([O, JC], F32, tag="sigsc")
            nc.sync.dma_start(sigsc[:], sigsc_d[:])
            sigbi = pers.tile([O, JC], F32, tag="sigbi")
            nc.sync.dma_start(sigbi[:], sigbi_d[:])

            # per-partition bias constants for the ACT Relu shifts
            bneg2 = pers.tile([128, 1], F32, tag="bneg2")
            nc.gpsimd.memset(bneg2[:], -0.2)
            bneg6 = pers.tile([128, 1], F32, tag="bneg6")
            nc.gpsimd.memset(bneg6[:], -0.6)

            # fp8 history: 8 tiles, each packing the 4 t-chunks of a j-pair
            xh_view = xh8_d.rearrange("(c p) j b -> p c (j b)", c=4)
            xts = []
            for r in range(8):
                xt = xhp.tile([128, 4, 512], FP8, tag="xh", name=f"xh{r}")
                nc.sync.dma_start(xt[:], xh_view[:, :, r * 512:(r + 1) * 512])
                xts.append(xt)

            # diagonal coef tiles, streamed in j order (combine chases this)
            diag = pers.tile([128, JC * NP * 128], F16, tag="diag")
            DGCH = NP * 128
            for j in range(JC):
                nc.sync.dma_start(diag[:, j * DGCH:(j + 1) * DGCH],
                                  diag_d[:, j * DGCH:(j + 1) * DGCH])

            def dg(p, j):
                off = (j * NP + p) * 128
                return diag[:, off:off + 128]

            # ---------------- feature buffers per half: [128, 5 x 2048] fp16
            R = [pers.tile([128, NBLK * HW], F16, tag=f"R{h}", name=f"R{h}")
                 for h in range(2)]
            Q = [pers.tile([128, NBLK * HW], F16, tag=f"Q{h}", name=f"Q{h}")
                 for h in range(2)]
            C = [pers.tile([128, NBLK * HW], F16, tag=f"C{h}", name=f"C{h}")
                 for h in range(2)]

            def blk(buf, h, name, n=1, q=None):
                o = BLK[name] * HW
                if q is not None:            # quarter slice within the block
                    o += (q % 2) * QW
                    return buf[h][:, o:o + QW]
                return buf[h][:, o:o + n * HW]

            # ---------------- PE: x_lagged (bf16, K=11) -> quarter psum ring
            # quarter qq covers j = 4qq..4qq+3; clip chases each quarter.
            for qq in range(4):
                pt = psq.tile([128, QW], F32, tag="q", name=f"xl{qq}")
                for jl in range(JQ):
                    ja = qq * JQ + jl
                    nc.tensor.matmul(pt[:, jl * B:(jl + 1) * B],
                                     wlag_sb[:, ja, :], win_sb[:, ja, :],
                                     start=True, stop=True)
                h = qq // 2
                nc.vector.tensor_scalar(blk(R, h, "xc", q=qq), pt[:],
                                        -1.0, 1.0, op0=ALU.max, op1=ALU.min)

            # ---------------- PE: mean stream (fp8, rows broadcast), quarters
            pms = []
            for qq in range(4):
                pm = psq.tile([128, QW], F32, tag="q", name=f"pm{qq}")
                pms.append(pm)
                for rr in range(2):
                    r = qq * 2 + rr
                    for ch in range(4):
                        nc.tensor.matmul(pm[:, rr * 512:(rr + 1) * 512],
                                         ones8[:], xts[r][:, ch, :],
                                         start=(ch == 0), stop=(ch == 3))

            # ---------------- features (halves interleaved so no engine
            # queue blocks waiting on a cross-engine dependency)
            # DVE: all shifts first (both halves), then x3/squares/cubes in
            # an order that trails the ACT square pipeline.
            negx = [None, None]
            for h in range(2):
                xc = blk(R, h, "xc")
                nc.vector.tensor_scalar(blk(R, h, "r1"), xc, -0.2, 0.0,
                                        op0=ALU.add, op1=ALU.max)
                nc.vector.tensor_scalar(blk(R, h, "r2"), xc, -0.6, 0.0,
                                        op0=ALU.add, op1=ALU.max)
                negx[h] = blk(Q, h, "r3")   # scratch (overwritten below)
                nc.vector.tensor_scalar(negx[h], xc, -1.0, None, op0=ALU.mult)
                nc.vector.tensor_scalar(blk(R, h, "r3"), negx[h], -0.2, 0.0,
                                        op0=ALU.add, op1=ALU.max)
                nc.vector.tensor_scalar(blk(R, h, "r4"), negx[h], -0.6, 0.0,
                                        op0=ALU.add, op1=ALU.max)
            # ACT queue: squares of xc (feeds x3 + combine p2) then r1|r2,
            # for both halves, ahead of the sigmoids.
            for h in range(2):
                nc.scalar.activation(blk(Q, h, "xc"), blk(R, h, "xc"),
                                     ACTF.Square)
            for h in range(2):
                nc.scalar.activation(blk(Q, h, "r1", 2), blk(R, h, "r1", 2),
                                     ACTF.Square)
            # DVE: x3, q[r3|r4], cubes — interleaved across halves so each
            # op's inputs are ready by the time the queue reaches it.
            for h in range(2):
                nc.vector.tensor_tensor(blk(C, h, "xc"), blk(Q, h, "xc"),
                                        blk(R, h, "xc"), op=ALU.mult)   # x3
                nc.vector.tensor_tensor(blk(Q, h, "r3", 2), blk(R, h, "r3", 2),
                                        blk(R, h, "r3", 2), op=ALU.mult)
            for h in range(2):
                nc.vector.tensor_tensor(blk(C, h, "r1", 2), blk(Q, h, "r1", 2),
                                        blk(R, h, "r1", 2), op=ALU.mult)
                nc.vector.tensor_tensor(blk(C, h, "r3", 2), blk(Q, h, "r3", 2),
                                        blk(R, h, "r3", 2), op=ALU.mult)

            # ---------------- ACT: sigmoids from mean PSUM -> alpha fp16
            alpha = pers.tile([128, JC * B], F16, tag="alpha")
            for qq in range(4):
                for jl in range(JQ):
                    ja = qq * JQ + jl
                    nc.scalar.activation(
                        alpha[:, ja * B:(ja + 1) * B],
                        pms[qq][:, jl * B:(jl + 1) * B],
                        ACTF.Sigmoid, bias=sigbi[:, ja:ja + 1],
                        scale=sigsc[:, ja:ja + 1])

            # ---------------- PE: combine (phase A then phase B) + z
            # p -> feature: 0 ones, 1 xc, 2 x2 (Q[xc]), 3 x3 (C[xc]),
            #               4 C[r1], 5 C[r2], 6 C[r3], 7 C[r4]
            def feat(p, ja):
                h, jl = ja // JH, ja % JH
                sl = slice(jl * B, (jl + 1) * B)
                if p == 0:
                    return ones16[:]
                src = {1: (R, "xc"), 2: (Q, "xc"), 3: (C, "xc"),
                       4: (C, "r1"), 5: (C, "r2"), 6: (C, "r3"),
                       7: (C, "r4")}[p]
                return blk(src[0], h, src[1])[:, sl]

            zb = pers.tile([128, JC * B], F16, tag="zb")
            for t in range(4):                      # 4 j's per 2-bank tile
                yt = psy.tile([128, 4 * B], F32, tag="y", name=f"y{t}")
                for hh in range(4):                 # contiguous 8-MM group
                    ja = 4 * t + hh
                    for p in range(8):
                        nc.tensor.matmul(yt[:, hh * B:(hh + 1) * B],
                                         dg(p, ja), feat(p, ja),
                                         start=(p == 0), stop=(p == 7))
                nc.vector.tensor_tensor(
                    zb[:, t * 1024:(t + 1) * 1024], yt[:],
                    alpha[:, t * 1024:(t + 1) * 1024], op=ALU.mult)

            # ---------------- j-sum: halving tree on DVE (small, late)
            t1 = pers.tile([128, 2048], F16, tag="t1")
            t2 = pers.tile([128, 1024], F16, tag="t2")
            t3 = pers.tile([128, 512], F16, tag="t3")
            acc = pers.tile([128, B], F32, tag="acc")
            nc.vector.tensor_tensor(t1[:], zb[:, 0:2048], zb[:, 2048:4096],
                                    op=ALU.add)
            nc.vector.tensor_tensor(t2[:], t1[:, 0:1024], t1[:, 1024:2048],
                                    op=ALU.add)
            nc.vector.tensor_tensor(t3[:], t2[:, 0:512], t2[:, 512:1024],
                                    op=ALU.add)
            nc.vector.tensor_tensor(acc[:], t3[:, 0:B], t3[:, B:2 * B],
                                    op=ALU.add)

            nc.sync.dma_start(out_d[:], acc[:])

    nc.compile()
    return nc


_CACHED_NC = None


def _get_program():
    global _CACHED_NC
    if _CACHED_NC is None:
        _CACHED_NC = _build_program()
    return _CACHED_NC


# ------------------------------------------------------------------ entry
def kernel(x_history, coef, lag_logits, mod_w, mod_b, adj_logits):
    in_maps = _host_precompute(x_history, coef, lag_logits, mod_w, mod_b,
                               adj_logits)
    nc = _get_program()
    res = bass_utils.run_bass_kernel_spmd(nc, in_maps,
                                          core_ids=list(range(NCORES)))
    total = np.zeros((O, B), dtype=np.float64)
    for c in range(NCORES):
        total += np.asarray(res.results[c]["outp"], dtype=np.float64)
    return np.ascontiguousarray(total.T.astype(np.float32))


# -------------------------------------------- pure-numpy emulation (testing)
def emulate(x_history, coef, lag_logits, mod_w, mod_b, adj_logits):
    """Numpy mirror of the v3.1 device algorithm (f32-ish, no dtype sim)."""
    in_maps = _host_precompute(x_history, coef, lag_logits, mod_w, mod_b,
                               adj_logits)
    total = np.zeros((O, B), dtype=np.float64)
    for c in range(NCORES):
        total += emulate_core(in_maps[c])
    return total.T.astype(np.float32)


def emulate_core(im):
    win = im["win"].astype(np.float64)            # [L,JC,B]
    wlg = im["wlag"].astype(np.float64)           # [L,JC,O]
    dgf = im["diag"].astype(np.float64).reshape(128, JC, NP, 128)
    params = dgf[np.arange(128), :, :, np.arange(128)]   # [128,JC,NP] (o,j,p)
    params = params.transpose(0, 2, 1)                   # [128,NP,JC]
    xm = im["xh8"].astype(np.float64).mean(axis=0)       # [JC,B]
    sigsc = im["sigsc"].astype(np.float64)        # [O,JC]
    sigbi = im["sigbi"].astype(np.float64)

    part = np.zeros((O, B), dtype=np.float64)
    for jl in range(JC):
        xl = wlg[:, jl, :].T @ win[:, jl, :]      # [O,B]
        x = np.clip(xl, -1.0, 1.0)
        f = [np.ones_like(x), x, x * x, x ** 3,
             np.maximum(x - 0.2, 0) ** 3, np.maximum(x - 0.6, 0) ** 3,
             np.maximum(-x - 0.2, 0) ** 3, np.maximum(-x - 0.6, 0) ** 3]
        y = np.zeros_like(x)
        for p in range(NP):
            y += params[:, p, jl][:, None] * f[p]
        lin = sigsc[:, jl][:, None] * (xm[jl] * T)[None, :] + sigbi[:, jl][:, None]
        part += y / (1.0 + np.exp(-lin))
    return part


# revision 19
# speedup vs baseline: 3.2707x; 1.0144x over previous
"""Trainium2 Bass kernel for nn_CDKANLayer (v3.1).

Computation (see problem reference):
  w_lag   = softmax(lag_logits, -1)                       [O,I,11]
  window  = x_history[:, T-11:T, :] reversed              [B,11,I]
  x_lagged[b,i,j] = sum_l window[b,l,j] * w_lag[i,j,l]
  xc      = clip(x_lagged, -1, 1)
  y_edge  = sum_c b_splines(xc) * coef                    (cubic B-spline)
  alpha   = sigmoid(mean_t(x_history)[b,j]*mod_w[i,j] + mod_b[i,j])
  out[b,i]= sum_j y_edge * alpha * sigmoid(adj_logits)[i,j]

v3.1 design (8 cores, shard in-features j; each core: 16 j x full B=256):
  - Two-sided truncated-power cubic (v2 param): features 1, x, x2, x3,
    r1^3, r2^3, r3^3, r4^3 with r = relu(+-x - t), negative-side signs
    folded into the host coefficients.
  - Combine on PE as accumulating diag matmuls, split into phase A
    (ones/x/x2/x3 — available early) and phase B (the four cubes), with
    interleaved PSUM groups across the 8 banks so PE never waits on the
    cube chain.
  - Feature ops at half-j granularity ([128,2048]) to shorten the
    clip->shift->square->cube critical path; split across DVE and ACT.
    GpSimd only does the small late j-sum tree (it cannot touch PSUM and
    big GpSimd ops destroy DVE throughput via shared SBUF ports).
  - z = y*alpha per j-pair on DVE; j-sum as halving tree; alpha fp16.
  - PSUM: quarter tiles [128,1024] ring (lag then mean) + 4 y banks.
"""

import os
import sys

import ml_dtypes
import numpy as np

for _p in ("/opt/trn_rl_repo", "/root/.axon_site/_ro/trn_rl_repo"):
    if os.path.isdir(_p) and _p not in sys.path:
        sys.path.insert(0, _p)

import concourse.bass as bass  # noqa: E402
import concourse.tile as tile  # noqa: E402
from concourse import bacc, mybir  # noqa: E402
from concourse import bass_utils  # noqa: E402

# ---------------------------------------------------------------- constants
B, T, I, O = 256, 512, 128, 128
L = 11                      # MAX_LAG + 1 lag taps
NCORES = 8
JC = I // NCORES            # j's per core = 16
JH = JC // 2                # j's per half = 8
JQ = JC // 4                # j's per quarter = 4
HW = JH * B                 # half width in columns = 2048
QW = JQ * B                 # quarter width = 1024
GRID_SIZE, SPLINE_ORDER = 5, 3
GRID_LO, GRID_HI = -1.0, 1.0
H = (GRID_HI - GRID_LO) / GRID_SIZE
NP = 8                      # combine terms: 1, x, x2, x3, c1, c2, c3, c4

F32 = mybir.dt.float32
F16 = mybir.dt.float16
BF16 = mybir.dt.bfloat16
FP8 = mybir.dt.float8e4
ALU = mybir.AluOpType
ACTF = mybir.ActivationFunctionType

NP_F16 = np.float16
NP_BF16 = ml_dtypes.bfloat16
NP_FP8 = ml_dtypes.float8_e4m3

# feature-block order inside R / Q / C buffers (per half):
#   r1 = relu(x-0.2), r2 = relu(x-0.6), r3 = relu(-x-0.2), r4 = relu(-x-0.6)
#   xc = clip(x)  (basis for x2/x3 in Q/C)
BLK = {"r1": 0, "r2": 1, "r3": 2, "r4": 3, "xc": 4}
NBLK = 5


# ------------------------------------------------------- host-side spline math
def _b_splines_np(x):
    """float64 copy of the reference b_splines (incl. its 1e-8 epsilons)."""
    g = (np.arange(-SPLINE_ORDER, GRID_SIZE + SPLINE_ORDER + 1, dtype=np.float64)
         * H + GRID_LO)
    x = np.asarray(x, dtype=np.float64)[..., None]
    bases = ((x >= g[:-1]) & (x < g[1:])).astype(np.float64)
    for i in range(1, SPLINE_ORDER + 1):
        t1 = (x - g[: -(i + 1)]) / (g[i:-1] - g[: -(i + 1)] + 1e-8) * bases[..., :-1]
        t2 = (g[i + 1:] - x) / (g[i + 1:] - g[1:-i] + 1e-8) * bases[..., 1:]
        bases = t1 + t2
    return bases


def _segment_poly_mats():
    """A[s] (4x8): on segment s, sum_c coef_c*B_c(x) = sum_d x^d*(A[s][d]@coef)."""
    mats = []
    for s in range(GRID_SIZE):
        lo = GRID_LO + s * H
        pts = lo + H * np.array([0.125, 0.375, 0.625, 0.875])
        Bm = _b_splines_np(pts)                       # [4, 8]
        V = np.vander(pts, 4, increasing=True)        # [4, 4]
        mats.append(np.linalg.solve(V, Bm))           # [4, 8]
    return np.stack(mats)                             # [5, 4, 8]


def _two_sided_params(coef64, mask):
    """[O, I, 8] float64: c0..c3 (center cubic), dR1,dR2,dL1,dL2 (r-form)."""
    Am = _segment_poly_mats()                          # [5,4,8]
    a = np.einsum("sdc,oic->sdoi", Am, coef64)         # [5,4,O,I]
    p = np.empty((O, I, NP), dtype=np.float64)
    p[..., 0:4] = np.moveaxis(a[2], 0, -1)             # center cubic c0..c3
    p[..., 4] = a[3, 3] - a[2, 3]                      # jump at +0.2
    p[..., 5] = a[4, 3] - a[3, 3]                      # jump at +0.6
    p[..., 6] = -(a[1, 3] - a[2, 3])                   # knot -0.2, relu(-x-.2)^3
    p[..., 7] = -(a[0, 3] - a[1, 3])                   # knot -0.6, relu(-x-.6)^3
    return p * mask[..., None]


def _host_precompute(x_history, coef, lag_logits, mod_w, mod_b, adj_logits):
    xh = np.asarray(x_history, dtype=np.float32)
    coef64 = np.asarray(coef, dtype=np.float64)
    ll = np.asarray(lag_logits, dtype=np.float64)

    m = ll.max(axis=-1, keepdims=True)
    e = np.exp(ll - m)
    w_lag = e / e.sum(axis=-1, keepdims=True)          # [O,I,L] f64

    mask = 1.0 / (1.0 + np.exp(-np.asarray(adj_logits, np.float64)[:O, :I]))
    params = _two_sided_params(coef64, mask)           # [O,I,8]

    window = xh[:, T - L:T, :][:, ::-1, :]             # [B,L,I]
    xh_tjb = np.ascontiguousarray(xh.transpose(1, 2, 0))  # [T, I, B]
    xh8_full = xh_tjb.astype(NP_FP8)

    rng = np.arange(128)
    in_maps = []
    for c in range(NCORES):
        sl = slice(c * JC, (c + 1) * JC)
        win = np.ascontiguousarray(
            window[:, :, sl].transpose(1, 2, 0)).astype(NP_BF16)   # [L,JC,B]
        wlg = np.ascontiguousarray(
            w_lag[:, sl, :].transpose(2, 1, 0)).astype(NP_BF16)    # [L,JC,O]
        xh8 = np.ascontiguousarray(xh8_full[:, sl, :])             # [T,JC,B]
        # diagonal combine tiles: [128 rows, j, p, 128 cols] j-major
        dg = np.zeros((128, JC, NP, 128), dtype=NP_F16)
        dg[rng, :, :, rng] = params[:, sl, :]                      # [O,JC,NP]
        sigsc = np.ascontiguousarray(
            np.asarray(mod_w, np.float64)[:, sl] / T).astype(np.float32)
        sigbi = np.ascontiguousarray(
            np.asarray(mod_b, np.float64)[:, sl]).astype(np.float32)
        in_maps.append({
            "win": win,
            "wlag": wlg,
            "xh8": xh8,
            "diag": np.ascontiguousarray(dg.reshape(128, NP * JC * 128)),
            "ones16": np.ones((128, B), dtype=NP_F16),
            "ones8": np.ones((128, 128), dtype=NP_FP8),
            "sigsc": sigsc,
            "sigbi": sigbi,
        })
    return in_maps


# ------------------------------------------------------------- device program
def _build_program():
    nc = bacc.Bacc("TRN2", target_bir_lowering=False, debug=False,
                   num_devices=NCORES)

    win_d = nc.dram_tensor("win", [L, JC, B], BF16, kind="ExternalInput")
    wlag_d = nc.dram_tensor("wlag", [L, JC, O], BF16, kind="ExternalInput")
    xh8_d = nc.dram_tensor("xh8", [T, JC, B], FP8, kind="ExternalInput")
    diag_d = nc.dram_tensor("diag", [128, NP * JC * 128], F16,
                            kind="ExternalInput")
    ones16_d = nc.dram_tensor("ones16", [128, B], F16, kind="ExternalInput")
    ones8_d = nc.dram_tensor("ones8", [128, 128], FP8, kind="ExternalInput")
    sigsc_d = nc.dram_tensor("sigsc", [O, JC], F32, kind="ExternalInput")
    sigbi_d = nc.dram_tensor("sigbi", [O, JC], F32, kind="ExternalInput")
    out_d = nc.dram_tensor("outp", [O, B], F32, kind="ExternalOutput")

    with tile.TileContext(nc) as tc:
        with (
            tc.tile_pool(name="pers", bufs=1) as pers,
            tc.tile_pool(name="xhp", bufs=4) as xhp,
            tc.tile_pool(name="psq", bufs=2, space=bass.MemorySpace.PSUM) as psq,
            tc.tile_pool(name="psy", bufs=2, space=bass.MemorySpace.PSUM) as psy,
        ):
            # ---------------- persistent loads (order = DMA priority)
            win_sb = pers.tile([L, JC, B], BF16, tag="win")
            nc.sync.dma_start(win_sb[:], win_d[:])
            wlag_sb = pers.tile([L, JC, O], BF16, tag="wlag")
            nc.sync.dma_start(wlag_sb[:], wlag_d[:])
            ones16 = pers.tile([128, B], F16, tag="ones16")
            nc.sync.dma_start(ones16[:], ones16_d[:])
            ones8 = pers.tile([128, 128], FP8, tag="ones8")
            nc.sync.dma_start(ones8[:], ones8_d[:])
            sigsc = pers.tile([O, JC], F32, tag="sigsc")
            nc.sync.dma_start(sigsc[:], sigsc_d[:])
            sigbi = pers.tile([O, JC], F32, tag="sigbi")
            nc.sync.dma_start(sigbi[:], sigbi_d[:])

            # per-partition bias constants for the ACT Relu shifts
            bneg2 = pers.tile([128, 1], F32, tag="bneg2")
            nc.gpsimd.memset(bneg2[:], -0.2)
            bneg6 = pers.tile([128, 1], F32, tag="bneg6")
            nc.gpsimd.memset(bneg6[:], -0.6)

            # fp8 history: 8 tiles, each packing the 4 t-chunks of a j-pair
            xh_view = xh8_d.rearrange("(c p) j b -> p c (j b)", c=4)
            xts = []
            for r in range(8):
                xt = xhp.tile([128, 4, 512], FP8, tag="xh", name=f"xh{r}")
                nc.sync.dma_start(xt[:], xh_view[:, :, r * 512:(r + 1) * 512])
                xts.append(xt)

            # diagonal coef tiles, streamed in j order (combine chases this)
            diag = pers.tile([128, JC * NP * 128], F16, tag="diag")
            DGCH = NP * 128
            for j in range(JC):
                nc.sync.dma_start(diag[:, j * DGCH:(j + 1) * DGCH],
                                  diag_d[:, j * DGCH:(j + 1) * DGCH])

            def dg(p, j):
                off = (j * NP + p) * 128
                return diag[:, off:off + 128]

            # ---------------- feature buffers per half: [128, 5 x 2048] fp16
            R = [pers.tile([128, NBLK * HW], F16, tag=f"R{h}", name=f"R{h}")
                 for h in range(2)]
            Q = [pers.tile([128, NBLK * HW], F16, tag=f"Q{h}", name=f"Q{h}")
                 for h in range(2)]
            C = [pers.tile([128, NBLK * HW], F16, tag=f"C{h}", name=f"C{h}")
                 for h in range(2)]

            def blk(buf, h, name, n=1, q=None):
                o = BLK[name] * HW
                if q is not None:            # quarter slice within the block
                    o += (q % 2) * QW
                    return buf[h][:, o:o + QW]
                return buf[h][:, o:o + n * HW]

            # ---------------- PE: x_lagged (bf16, K=11) -> quarter psum ring
            # quarter qq covers j = 4qq..4qq+3; clip chases each quarter.
            for qq in range(4):
                pt = psq.tile([128, QW], F32, tag="q", name=f"xl{qq}")
                for jl in range(JQ):
                    ja = qq * JQ + jl
                    nc.tensor.matmul(pt[:, jl * B:(jl + 1) * B],
                                     wlag_sb[:, ja, :], win_sb[:, ja, :],
                                     start=True, stop=True)
                h = qq // 2
                nc.vector.tensor_scalar(blk(R, h, "xc", q=qq), pt[:],
                                        -1.0, 1.0, op0=ALU.max, op1=ALU.min)

            # ---------------- PE: mean stream (fp8, rows broadcast), quarters
            pms = []
            for qq in range(4):
                pm = psq.tile([128, QW], F32, tag="q", name=f"pm{qq}")
                pms.append(pm)
                for rr in range(2):
                    r = qq * 2 + rr
                    for ch in range(4):
                        nc.tensor.matmul(pm[:, rr * 512:(rr + 1) * 512],
                                         ones8[:], xts[r][:, ch, :],
                                         start=(ch == 0), stop=(ch == 3))

            # ---------------- features (halves interleaved so no engine
            # queue blocks waiting on a cross-engine dependency)
            # DVE: all shifts first (both halves), then x3/squares/cubes in
            # an order that trails the ACT square pipeline.
            negx = [None, None]
            for h in range(2):
                xc = blk(R, h, "xc")
                nc.vector.tensor_scalar(blk(R, h, "r1"), xc, -0.2, 0.0,
                                        op0=ALU.add, op1=ALU.max)
                nc.vector.tensor_scalar(blk(R, h, "r2"), xc, -0.6, 0.0,
                                        op0=ALU.add, op1=ALU.max)
                negx[h] = blk(Q, h, "r3")   # scratch (overwritten below)
                nc.vector.tensor_scalar(negx[h], xc, -1.0, None, op0=ALU.mult)
                nc.vector.tensor_scalar(blk(R, h, "r3"), negx[h], -0.2, 0.0,
                                        op0=ALU.add, op1=ALU.max)
                nc.vector.tensor_scalar(blk(R, h, "r4"), negx[h], -0.6, 0.0,
                                        op0=ALU.add, op1=ALU.max)
            # ACT queue: squares interleaved with sigmoid quarters — the
            # sigmoids free the mean-PSUM ring slots so the mean matmuls
            # (and the combine queued behind them) aren't blocked.
            alpha = pers.tile([128, JC * B], F16, tag="alpha")

            def sig_quarter(qq):
                for jl in range(JQ):
                    ja = qq * JQ + jl
                    nc.scalar.activation(
                        alpha[:, ja * B:(ja + 1) * B],
                        pms[qq][:, jl * B:(jl + 1) * B],
                        ACTF.Sigmoid, bias=sigbi[:, ja:ja + 1],
                        scale=sigsc[:, ja:ja + 1])

            nc.scalar.activation(blk(Q, 0, "xc"), blk(R, 0, "xc"), ACTF.Square)
            sig_quarter(0)
            nc.scalar.activation(blk(Q, 1, "xc"), blk(R, 1, "xc"), ACTF.Square)
            sig_quarter(1)
            nc.scalar.activation(blk(Q, 0, "r1", 2), blk(R, 0, "r1", 2),
                                 ACTF.Square)
            sig_quarter(2)
            nc.scalar.activation(blk(Q, 1, "r1", 2), blk(R, 1, "r1", 2),
                                 ACTF.Square)
            sig_quarter(3)
            # DVE: x3, q[r3|r4], cubes — interleaved across halves so each
            # op's inputs are ready by the time the queue reaches it.
            for h in range(2):
                nc.vector.tensor_tensor(blk(C, h, "xc"), blk(Q, h, "xc"),
                                        blk(R, h, "xc"), op=ALU.mult)   # x3
                nc.vector.tensor_tensor(blk(Q, h, "r3", 2), blk(R, h, "r3", 2),
                                        blk(R, h, "r3", 2), op=ALU.mult)
            for h in range(2):
                nc.vector.tensor_tensor(blk(C, h, "r1", 2), blk(Q, h, "r1", 2),
                                        blk(R, h, "r1", 2), op=ALU.mult)
                nc.vector.tensor_tensor(blk(C, h, "r3", 2), blk(Q, h, "r3", 2),
                                        blk(R, h, "r3", 2), op=ALU.mult)

            # ---------------- PE: combine + z
            # p -> feature: 0 ones, 1 xc, 2 x2 (Q[xc]), 3 x3 (C[xc]),
            #               4 C[r1], 5 C[r2], 6 C[r3], 7 C[r4]
            def feat(p, ja):
                h, jl = ja // JH, ja % JH
                sl = slice(jl * B, (jl + 1) * B)
                if p == 0:
                    return ones16[:]
                src = {1: (R, "xc"), 2: (Q, "xc"), 3: (C, "xc"),
                       4: (C, "r1"), 5: (C, "r2"), 6: (C, "r3"),
                       7: (C, "r4")}[p]
                return blk(src[0], h, src[1])[:, sl]

            zb = pers.tile([128, JC * B], F16, tag="zb")
            for t in range(4):                      # 4 j's per 2-bank tile
                yt = psy.tile([128, 4 * B], F32, tag="y", name=f"y{t}")
                for hh in range(4):                 # contiguous 8-MM group
                    ja = 4 * t + hh
                    for p in range(8):
                        nc.tensor.matmul(yt[:, hh * B:(hh + 1) * B],
                                         dg(p, ja), feat(p, ja),
                                         start=(p == 0), stop=(p == 7))
                nc.vector.tensor_tensor(
                    zb[:, t * 1024:(t + 1) * 1024], yt[:],
                    alpha[:, t * 1024:(t + 1) * 1024], op=ALU.mult)

            # ---------------- j-sum: halving tree on DVE (small, late)
            t1 = pers.tile([128, 2048], F16, tag="t1")
            t2 = pers.tile([128, 1024], F16, tag="t2")
            t3 = pers.tile([128, 512], F16, tag="t3")
            acc = pers.tile([128, B], F32, tag="acc")
            nc.vector.tensor_tensor(t1[:], zb[:, 0:2048], zb[:, 2048:4096],
                                    op=ALU.add)
            nc.vector.tensor_tensor(t2[:], t1[:, 0:1024], t1[:, 1024:2048],
                                    op=ALU.add)
            nc.vector.tensor_tensor(t3[:], t2[:, 0:512], t2[:, 512:1024],
                                    op=ALU.add)
            nc.vector.tensor_tensor(acc[:], t3[:, 0:B], t3[:, B:2 * B],
                                    op=ALU.add)

            nc.sync.dma_start(out_d[:], acc[:])

    nc.compile()
    return nc


_CACHED_NC = None


def _get_program():
    global _CACHED_NC
    if _CACHED_NC is None:
        _CACHED_NC = _build_program()
    return _CACHED_NC


# ------------------------------------------------------------------ entry
def kernel(x_history, coef, lag_logits, mod_w, mod_b, adj_logits):
    in_maps = _host_precompute(x_history, coef, lag_logits, mod_w, mod_b,
                               adj_logits)
    nc = _get_program()
    res = bass_utils.run_bass_kernel_spmd(nc, in_maps,
                                          core_ids=list(range(NCORES)))
    total = np.zeros((O, B), dtype=np.float64)
    for c in range(NCORES):
        total += np.asarray(res.results[c]["outp"], dtype=np.float64)
    return np.ascontiguousarray(total.T.astype(np.float32))


# -------------------------------------------- pure-numpy emulation (testing)
def emulate(x_history, coef, lag_logits, mod_w, mod_b, adj_logits):
    """Numpy mirror of the v3.1 device algorithm (f32-ish, no dtype sim)."""
    in_maps = _host_precompute(x_history, coef, lag_logits, mod_w, mod_b,
                               adj_logits)
    total = np.zeros((O, B), dtype=np.float64)
    for c in range(NCORES):
        total += emulate_core(in_maps[c])
    return total.T.astype(np.float32)


def emulate_core(im):
    win = im["win"].astype(np.float64)            # [L,JC,B]
    wlg = im["wlag"].astype(np.float64)           # [L,JC,O]
    dgf = im["diag"].astype(np.float64).reshape(128, JC, NP, 128)
    params = dgf[np.arange(128), :, :, np.arange(128)]   # [128,JC,NP] (o,j,p)
    params = params.transpose(0, 2, 1)                   # [128,NP,JC]
    xm = im["xh8"].astype(np.float64).mean(axis=0)       # [JC,B]
    sigsc = im["sigsc"].astype(np.float64)        # [O,JC]
    sigbi = im["sigbi"].astype(np.float64)

    part = np.zeros((O, B), dtype=np.float64)
    for jl in range(JC):
        xl = wlg[:, jl, :].T @ win[:, jl, :]      # [O,B]
        x = np.clip(xl, -1.0, 1.0)
        f = [np.ones_like(x), x, x * x, x ** 3,
             np.maximum(x - 0.2, 0) ** 3, np.maximum(x - 0.6, 0) ** 3,
             np.maximum(-x - 0.2, 0) ** 3, np.maximum(-x - 0.6, 0) ** 3]
        y = np.zeros_like(x)
        for p in range(NP):
            y += params[:, p, jl][:, None] * f[p]
        lin = sigsc[:, jl][:, None] * (xm[jl] * T)[None, :] + sigbi[:, jl][:, None]
        part += y / (1.0 + np.exp(-lin))
    return part
